# revision 17
# baseline (speedup 1.0000x reference)
"""Trainium2 Bass kernel for nn_DictNet_44547400794580.

Math: the loss only needs each graph's embedding
    emb_g = (1/N) * (1 - w_g)^T X_g,   w_g = sum_f c_f * (40(L_g - b_f I)^4 + I)^(-2) @ 1
where L_g = I - Ahat_g (sym-normalized Laplacian) and c = C/||C||_2.
All 11 filters are fixed rational functions of Ahat_g (spectrum in [-1,1]), so
w_g = p(Ahat_g) @ 1 for a single degree-79 Chebyshev polynomial whose
coefficients are (fixed interpolation matrix) @ c.  Evaluated on-device with a
baby-step/giant-step scheme in the product basis T_r(x)*T_q(T_8(x)):
  - 3 matrix squarings build T_2, T_4, T_8 of Ahat
  - 8 baby vectors g_r = T_r(Ahat) @ 1 (via Chebyshev product identities)
  - 10-term giant chain in T_8 over the 8-column baby block
Sharding: data-parallel over graphs, 2 graphs per core on 8 cores.  The host
gathers the (tiny) [16,256] embeddings and does the final cdist/sparsity
reduction in float64 — the same index bookkeeping the reference itself
performs on the host with numpy.
"""
import sys
if '/opt/trn_rl_repo' not in sys.path:
    sys.path.insert(0, '/opt/trn_rl_repo')

import numpy as np

# ---------------------------------------------------------------------------
# problem constants (hardcoded per contract)
G, N, F, K, NF = 16, 512, 256, 4, 11
NCORES = 8
GPC = G // NCORES          # graphs per core
P = 128
NCH = N // P               # 512 = 4 partition chunks
DEG = 27                   # Chebyshev degree (end-to-end rel err ~5e-6 + fp32r noise)
S = 4                      # baby steps
MQ = DEG // S + 1          # giant columns q = 0..7
NG = S * MQ                # 32 product-basis coefficients

f32 = None  # set lazily (mybir import deferred so host-side use stays cheap)


# ---------------------------------------------------------------------------
# host-side fixed constants: Chebyshev coefficients of the 11 filters in the
# product basis, as a [NF, NG] matrix (pure math, no input data).
def _build_gamma_mat():
    bs = np.linspace(0.0, 2.0, NF)

    def psi(a, b):
        return (40.0 * (1.0 - a - b) ** 4 + 1.0) ** (-2)

    k = np.arange(DEG + 1)
    xk = np.cos(np.pi * (k + 0.5) / (DEG + 1))
    Mx = np.cos(k[:, None] * np.pi * (k[None, :] + 0.5) / (DEG + 1))

    gm = np.zeros((NF, NG))
    for fi, b in enumerate(bs):
        c = 2.0 / (DEG + 1) * (Mx @ psi(xk, b))
        c[0] *= 0.5
        beta = c.copy()
        gamma = np.zeros((S, MQ))
        for kk in range(DEG, S - 1, -1):
            q, r = divmod(kk, S)
            if r == 0:
                gamma[0, q] = beta[kk]
            else:
                gamma[r, q] = 2.0 * beta[kk]
                beta[S * q - r] -= beta[kk]
        for r in range(S):
            gamma[r, 0] += beta[r]
        # flatten q-major: index q*S + r
        gm[fi] = gamma.T.reshape(-1)
    return gm.astype(np.float32)


GAMMA_MAT = _build_gamma_mat()          # [11, 80]

TRACE = False
LAST_EXEC_NS = None
LAST_RESULTS = None


# ---------------------------------------------------------------------------
# device kernel (one core: GPC graphs)
#
# Row-form chain: vectors are the stationary matmul operand (cheap LDWEIGHTS),
# the matrix streams once per step; PE transposes flip row results back to
# column form for the next step's stationary operand.  w accumulates via per-q
# K=S matmuls into one persistent PSUM row; ||C|| normalization and the (1-w)
# affine fold into the final eviction.  Matrices stored pre-doubled where used
# doubled (ah2=2*Ahat, t4d=2*T4; exact power-of-2 scalings).
def build_device_kernel(tc, outs, ins):
    import concourse.mybir as mybir
    from concourse.masks import make_identity
    from contextlib import ExitStack

    nc = tc.nc
    dt = mybir.dt.float32
    dtr = mybir.dt.float32r
    Alu = mybir.AluOpType

    def mmr(out, lhsT, rhs, **kw):
        # float32r streams at full rate for N>=256 (fp32 pays 2 passes)
        nc.tensor.matmul(out, lhsT=lhsT.bitcast(dtr), rhs=rhs.bitcast(dtr), **kw)

    adj_d, x_d, c_d, g_d = ins
    emb_d = outs

    with ExitStack() as ctx:
        sb = ctx.enter_context(tc.tile_pool(name="sb", bufs=1))
        sb2 = ctx.enter_context(tc.tile_pool(name="sb2", bufs=2))

        # ---- constants
        identg = sb.tile([P, P], dt, tag="identg", name="identg")
        make_identity(nc, identg)
        identv = sb.tile([P, P], dt, tag="identv", name="identv")
        nc.vector.tensor_copy(identv.bitcast(dtr), identg)
        negI = sb.tile([P, P], dt, tag="negI", name="negI")
        nc.vector.tensor_scalar_mul(negI, identv, -1.0)
        negI2 = sb.tile([P, P], dt, tag="negI2", name="negI2")
        nc.vector.tensor_scalar_mul(negI2, identv, -2.0)
        ones_col = sb.tile([P, 1], dt, tag="ones_col", name="ones_col")
        nc.vector.tensor_scalar(ones_col.bitcast(dtr), identv[:, 0:1], 0.0, 1.0, Alu.mult, Alu.add)
        ones11 = sb.tile([NF, 1], dt, tag="ones11", name="ones11")
        nc.vector.memset(ones11, 1.0)

        # ---- gamma columns [S, MQ] (unnormalized) + rnorm = 1/||C||
        cvec = sb.tile([NF, 1], dt, tag="cvec", name="cvec")
        nc.sync.dma_start(cvec, c_d)
        gmat = sb.tile([NF, NG], dt, tag="gmat", name="gmat")
        nc.sync.dma_start(gmat, g_d)
        gamcol = sb.tile([S, MQ], dt, tag="gamcol", name="gamcol")
        with tc.tile_pool(name="psg", bufs=2, space="PSUM") as psg:
            csq = sb.tile([NF, 1], dt, tag="csq", name="csq")
            nc.vector.tensor_mul(csq, cvec, cvec)
            ps1 = psg.tile([1, 1], dt, tag="g1", name="g1")
            nc.tensor.matmul(ps1, lhsT=csq, rhs=ones11, start=True, stop=True)
            snorm = sb.tile([1, 1], dt, tag="snorm", name="snorm")
            nc.scalar.sqrt(snorm, ps1)
            rnorm = sb.tile([1, 1], dt, tag="rnorm", name="rnorm")
            nc.vector.reciprocal(rnorm, snorm)
            nrnorm = sb.tile([1, 1], dt, tag="nrnorm", name="nrnorm")
            nc.vector.tensor_scalar_mul(nrnorm, rnorm, -1.0)
            for q in range(MQ):
                psq = psg.tile([S, 1], dt, tag="gq", name="gq")
                nc.tensor.matmul(psq, lhsT=gmat[:, q * S:(q + 1) * S], rhs=cvec,
                                 start=True, stop=True)
                nc.vector.tensor_copy(gamcol[:, q:q + 1].bitcast(dtr), psq)

        # ---- per-graph tiles
        adj0 = {}
        xs = {}
        ah2 = {}
        t2 = {}
        t4d = {}
        for g in range(GPC):
            for kk in range(NCH):
                adj0[g, kk] = sb.tile([P, N], dt, tag=f"adj0_{g}_{kk}", name=f"adj0_{g}_{kk}")
                nc.sync.dma_start(adj0[g, kk], adj_d[g, kk * P:(kk + 1) * P, :])
        for g in range(GPC):
            x0 = sb.tile([P, NCH, F], dt, tag=f"xin_{g}", name=f"xin_{g}")
            nc.sync.dma_start(x0, x_d[g].rearrange("(c p) f -> p c f", p=P))
            for kk in range(NCH):
                xs[g, kk] = sb.tile([P, F], dt, tag=f"x{g}_{kk}", name=f"x{g}_{kk}")
                nc.scalar.mul(xs[g, kk].bitcast(dtr), x0[:, kk, :], 1.0 / N)

        with tc.tile_pool(name="psb", bufs=3, space="PSUM") as psb:
            # ---- degree + dinv: deg via fp32 matmul (PE is idle at startup),
            # dinv = |max(deg,1)|^(-1/2) in one ACT LUT op.  No zero-degree
            # mask needed: dinv only ever multiplies adj entries that are 0
            # on zero-degree rows/cols.
            dinv_row = {}
            d2row = {}
            for g in range(GPC):
                dps = psb.tile([1, N], dt, tag="row", name="row")
                for kk in range(NCH):
                    nc.tensor.matmul(dps, lhsT=ones_col, rhs=adj0[g, kk],
                                     start=(kk == 0), stop=(kk == NCH - 1))
                dmax = sb.tile([1, N], dt, tag=f"dmax{g}", name=f"dmax{g}")
                nc.vector.tensor_scalar_max(dmax, dps, 1.0)
                lndeg = sb.tile([1, N], dt, tag=f"lndeg{g}", name=f"lndeg{g}")
                nc.scalar.activation(lndeg, dmax, mybir.ActivationFunctionType.Ln)
                dinv_row[g] = sb.tile([1, N], dt, tag=f"dinv{g}", name=f"dinv{g}")
                nc.scalar.activation(dinv_row[g].bitcast(dtr), lndeg,
                                     mybir.ActivationFunctionType.Exp, scale=-0.5)
                d2row[g] = sb.tile([1, N], dt, tag=f"d2row{g}", name=f"d2row{g}")
                nc.vector.tensor_scalar_mul(d2row[g].bitcast(dtr), dinv_row[g], 2.0)

            # ---- ah2 = 2*Ahat
            for g in range(GPC):
                for kk in range(NCH):
                    dps = psb.tile([P, N], dt, tag="big", name="big")
                    mmr(dps, d2row[g][:, kk * P:(kk + 1) * P],
                        dinv_row[g], start=True, stop=True)
                    ah2[g, kk] = sb.tile([P, N], dt, tag=f"ah{g}_{kk}", name=f"ah{g}_{kk}")
                    nc.vector.tensor_tensor(ah2[g, kk].bitcast(dtr), adj0[g, kk], dps, Alu.mult)

            # ---- squarings: T2 = (ah2@ah2)/2 - I ; t4d = 4*T2@T2 - 2I
            def square_into(src_m, dst_map, g, name, scale, dI):
                for m in range(NCH):
                    ps = psb.tile([P, N], dt, tag="big", name="big")
                    for kk in range(NCH):
                        mmr(ps, src_m[g, kk][:, m * P:(m + 1) * P],
                            src_m[g, kk], start=(kk == 0), stop=(kk == NCH - 1))
                    t = sb.tile([P, N], dt, tag=f"{name}{g}_{m}", name=f"{name}{g}_{m}")
                    h = N // 2
                    nc.vector.tensor_scalar_mul(t[:, :h].bitcast(dtr), ps[:, :h], scale)
                    nc.scalar.mul(t[:, h:].bitcast(dtr), ps[:, h:], scale)
                    nc.vector.tensor_add(t[:, m * P:(m + 1) * P].bitcast(dtr), t[:, m * P:(m + 1) * P], dI)
                    dst_map[g, m] = t

            for g in range(GPC):
                square_into(ah2, t2, g, "t2", 0.5, negI)
            for g in range(GPC):
                square_into(t2, t4d, g, "t4", 4.0, negI2)

        # ---- vector phase
        with ExitStack() as vctx:
            psv = vctx.enter_context(tc.tile_pool(name="psv", bufs=3, space="PSUM"))
            psw = vctx.enter_context(tc.tile_pool(name="psw", bufs=1, space="PSUM"))

            # G and Z in column form: ONE [P, NCH*S] tile per graph,
            # columns kk*S + r  (chunk-major, baby/chain index minor)
            gcol = {}
            grow = {}
            wps = {}
            for g in range(GPC):
                gcol[g] = sb.tile([P, NCH * S], dt, tag=f"gc{g}", name=f"gc{g}")
                for kk in range(NCH):
                    nc.vector.tensor_scalar(gcol[g][:, kk * S:kk * S + 1].bitcast(dtr),
                                            identv[:, 0:1], 0.0, 1.0, Alu.mult, Alu.add)
                wps[g] = psw.tile([1, N], dt, tag=f"wps{g}", name=f"wps{g}")

            def row_matvec(mat, g, lhs_cols, out_ap, scale=None):
                nr = lhs_cols[0].shape[-1]
                ps = psv.tile([S, N], dt, tag="cr", name="cr")[:nr, :]
                for kk in range(NCH):
                    mmr(ps, lhs_cols[kk], mat[g, kk],
                        start=(kk == 0), stop=(kk == NCH - 1))
                if scale is None:
                    nc.vector.tensor_copy(out_ap, ps)
                elif scale == 'copy_r':
                    nc.vector.tensor_copy(out_ap.bitcast(dtr), ps)
                else:
                    nc.vector.tensor_scalar_mul(out_ap.bitcast(dtr), ps, scale)

            def transpose_row_batch(row_ap, nr):
                """row_ap [nr, N](SBUF) -> one [P, NCH*nr] psum (cols kk*nr + r)."""
                pst = psv.tile([P, NCH * S], dt, tag="tp", name="tp")
                for kk in range(NCH):
                    nc.tensor.transpose(pst[:, kk * nr:(kk + 1) * nr],
                                        row_ap[:, kk * P:(kk + 1) * P], identv[:nr, :nr])
                return pst

            # babies: g1 = (ah2 u)/2 ; g2 = t2 u ; g3 = ah2 g2 - g1
            r1 = {}
            r2 = {}
            for g in range(GPC):
                r1[g] = sb.tile([1, N], dt, tag=f"r1{g}", name=f"r1{g}")
                row_matvec(ah2, g, [ones_col] * NCH, r1[g], scale=0.5)
                r2[g] = sb.tile([1, N], dt, tag=f"r2{g}", name=f"r2{g}")
                row_matvec(t2, g, [ones_col] * NCH, r2[g], scale='copy_r')
            for g in range(GPC):
                pst = transpose_row_batch(r1[g], 1)
                nc.vector.tensor_copy(gcol[g][:, 1:NCH * S:S].bitcast(dtr), pst[:, :NCH])
                pst = transpose_row_batch(r2[g], 1)
                nc.vector.tensor_copy(gcol[g][:, 2:NCH * S:S].bitcast(dtr), pst[:, :NCH])
            for g in range(GPC):
                h3 = sb.tile([1, N], dt, tag=f"h3{g}", name=f"h3{g}")
                row_matvec(ah2, g, [gcol[g][:, kk * S + 2:kk * S + 3] for kk in range(NCH)],
                           h3, scale='copy_r')
                pst = transpose_row_batch(h3, 1)
                nc.vector.tensor_sub(gcol[g][:, 3:NCH * S:S].bitcast(dtr), pst[:, :NCH],
                                     gcol[g][:, 1:NCH * S:S])
            for g in range(GPC):
                grow[g] = sb.tile([S, N], dt, tag=f"gr{g}", name=f"gr{g}")
                pst = psv.tile([S, N], dt, tag="cr", name="cr")
                for kk in range(NCH):
                    nc.tensor.transpose(pst[:, kk * P:(kk + 1) * P],
                                        gcol[g][:, kk * S:(kk + 1) * S], identv)
                nc.vector.tensor_copy(grow[g].bitcast(dtr), pst)

            # giant chain + w accumulation
            zrow_prev = {}
            zrow_cur = {}
            zcol_cur = {}
            for g in range(GPC):
                zr = sb.tile([S, N], dt, tag=f"zr1_{g}", name=f"zr1_{g}")
                ps = psv.tile([S, N], dt, tag="cr", name="cr")
                for kk in range(NCH):
                    mmr(ps, gcol[g][:, kk * S:(kk + 1) * S], t4d[g, kk],
                        start=(kk == 0), stop=(kk == NCH - 1))
                nc.vector.tensor_scalar_mul(zr.bitcast(dtr), ps, 0.5)
                zrow_prev[g] = grow[g]
                zrow_cur[g] = zr
                pst = transpose_row_batch(zr, S)
                zc = sb.tile([P, NCH * S], dt, tag=f"zc1_{g}", name=f"zc1_{g}")
                nc.scalar.copy(zc.bitcast(dtr), pst)
                zcol_cur[g] = zc
                mmr(wps[g], gamcol[:, 0:1], grow[g],
                    start=True, stop=False, skip_group_check=True)
                mmr(wps[g], gamcol[:, 1:2], zr,
                    start=False, stop=False, skip_group_check=True)

            for q in range(2, MQ):
                last = (q == MQ - 1)
                for g in range(GPC):
                    ps = psv.tile([S, N], dt, tag="cr", name="cr")
                    for kk in range(NCH):
                        mmr(ps, zcol_cur[g][:, kk * S:(kk + 1) * S], t4d[g, kk],
                            start=(kk == 0), stop=(kk == NCH - 1))
                    zr = sb.tile([S, N], dt, tag=f"zrow{q % 3}_{g}", name=f"zrow{q % 3}_{g}")
                    nc.vector.tensor_sub(zr.bitcast(dtr), ps, zrow_prev[g])
                    zrow_prev[g] = zrow_cur[g]
                    zrow_cur[g] = zr
                    if not last:
                        pst = transpose_row_batch(zr, S)
                        zc = sb.tile([P, NCH * S], dt, tag=f"zcol{q % 2}_{g}", name=f"zcol{q % 2}_{g}")
                        nc.scalar.copy(zc.bitcast(dtr), pst)
                        zcol_cur[g] = zc
                    mmr(wps[g], gamcol[:, q:q + 1], zr,
                        start=False, stop=last, skip_group_check=True)

            # ---- v = 1 - rnorm*w ; emb = v^T (X/N)
            for g in range(GPC):
                vrow = sb.tile([1, N], dt, tag=f"vrow{g}", name=f"vrow{g}")
                nc.vector.tensor_scalar(vrow.bitcast(dtr), wps[g], nrnorm[:, 0:1], 1.0, Alu.mult, Alu.add)
                pst = transpose_row_batch(vrow, 1)
                vcol = sb.tile([P, NCH], dt, tag=f"vc{g}", name=f"vc{g}")
                nc.vector.tensor_copy(vcol.bitcast(dtr), pst[:, :NCH])
                pse = psv.tile([1, F], dt, tag="cr", name="cr")
                for kk in range(NCH):
                    mmr(pse, vcol[:, kk:kk + 1], xs[g, kk],
                        start=(kk == 0), stop=(kk == NCH - 1))
                erow = sb.tile([1, F], dt, tag=f"erow{g}", name=f"erow{g}")
                nc.vector.tensor_copy(erow, pse)
                nc.sync.dma_start(emb_d[g:g + 1, :], erow)


# ---------------------------------------------------------------------------
# host: final loss from embeddings (float64; same bookkeeping the reference
# does on the host with numpy: class index construction / product combos)
def final_loss(emb, C, y):
    from itertools import product as _product
    e = emb.astype(np.float64)
    sq = (e * e).sum(1)
    D2 = sq[:, None] + sq[None, :] - 2 * e @ e.T
    D = np.sqrt(np.maximum(D2, 0.0))
    np.fill_diagonal(D, 0.0)
    y = np.asarray(y)
    class_idx = [np.nonzero(y == i)[0] for i in range(K)]
    neg = np.array(list(_product(*class_idx)))
    h1 = -sum(D[np.ix_(cb, cb)].mean() for cb in neg)
    h2 = sum(D[np.ix_(ci, ci)].mean() for ci in class_idx)
    beta = neg.shape[0] / K
    C64 = np.asarray(C, np.float64)
    dims = np.sqrt(float(C64.shape[0]))
    l1 = np.abs(C64).sum(0)
    l2 = np.sqrt((C64 * C64).sum(0))
    sparsity = np.mean((dims - l1 / l2) / (dims - 1))
    return sparsity + h2 + h1 / beta


# ---------------------------------------------------------------------------
_COMPILED = {}


def _get_nc():
    if "nc" in _COMPILED:
        return _COMPILED["nc"]
    import concourse.mybir as mybir
    import concourse.tile as tile
    from concourse import bacc

    dt = mybir.dt.float32
    nc = bacc.Bacc("TRN2", target_bir_lowering=False, debug=False)
    adj_d = nc.dram_tensor("adj", [GPC, N, N], dt, kind="ExternalInput").ap()
    x_d = nc.dram_tensor("x", [GPC, N, F], dt, kind="ExternalInput").ap()
    c_d = nc.dram_tensor("cvec", [NF, 1], dt, kind="ExternalInput").ap()
    g_d = nc.dram_tensor("gmat", [NF, NG], dt, kind="ExternalInput").ap()
    emb_d = nc.dram_tensor("emb", [GPC, F], dt, kind="ExternalOutput").ap()

    with tile.TileContext(nc) as tc:
        build_device_kernel(tc, emb_d, (adj_d, x_d, c_d, g_d))
    nc.compile()

    _COMPILED["nc"] = nc
    return nc


def kernel(adj, x, C, y):
    global LAST_EXEC_NS, LAST_RESULTS
    from concourse.bass_utils import run_bass_kernel_spmd

    adj = np.ascontiguousarray(np.asarray(adj, np.float32))
    x = np.ascontiguousarray(np.asarray(x, np.float32))
    C = np.ascontiguousarray(np.asarray(C, np.float32))

    nc = _get_nc()
    in_maps = []
    for c in range(NCORES):
        in_maps.append({
            "adj": adj[c * GPC:(c + 1) * GPC],
            "x": x[c * GPC:(c + 1) * GPC],
            "cvec": C,
            "gmat": GAMMA_MAT,
        })
    res = run_bass_kernel_spmd(nc, in_maps, core_ids=list(range(NCORES)), trace=TRACE)
    LAST_EXEC_NS = res.exec_time_ns
    LAST_RESULTS = res
    emb = np.concatenate([res.results[c]["emb"] for c in range(NCORES)], axis=0)
    loss = final_loss(emb, C, y)
    return np.float32(loss)
